# revision 1
# baseline (speedup 1.0000x reference)
"""Trainium2 Bass kernel for nn_ChannelSpatialModulatedConv2d.

Data-parallel over batch across 8 NeuronCores (4 samples each). Per core:
  1. style  = style_chan @ (mod_w*ls).T + mod_b             (PE, fp32)
  2. wsc    = conv_scale * weight * style[b,ci]             (DVE tensor_scalar)
     demod  = rsqrt(sum(wsc^2) over (ci,kk) + eps) per co   (ACT square, DVE
              kk-reduce, PE ones-matmul -> [128co,1] partition-native)
  3. conv2d(x[b], wsc) via 18 accumulating fp32r matmuls per [128co x 512yx]
     PSUM tile (2 ci-tiles x 9 shifts), shifted-window APs over a zero-padded
     66x66 SBUF image.
  4. sp map = style_sp @ (sp_w*ls).T + sp_b, spatially demodulated. The PSUM
     epilogue computes out = (psum * demod[co]) * spmap[yx] in one fused
     scalar_tensor_tensor (demod is NOT folded into the weights; conv is
     linear so this is equivalent).

The baked walrus build only supports ONE sync wait per instruction, so the
Bass subclass rewrites the scheduled BIR JSON, hoisting extra waits onto
single-wait EventSemaphore carriers inserted before the instruction (same
engine => identical blocking semantics).
"""

import json
import sys
from contextlib import ExitStack

for _p in ("/opt/pypackages", "/opt/trn_rl_repo"):
    if _p not in sys.path:
        sys.path.insert(0, _p)

import numpy as np

import concourse.bass as bass
import concourse.mybir as mybir
import concourse.tile as tile
from concourse.tile_rust import add_dep_helper
from concourse.bass_utils import run_bass_kernel_spmd

# Problem constants (hardcoded per harness contract)
B, CIN, COUT, K = 32, 256, 256, 3
STYLE_DIM, SP = 512, 64
EPS = 1e-6
LS = 1.0 / (STYLE_DIM // 2) ** 0.5      # EqualLinear scale = 1/16
CS = 1.0 / (CIN * K * K) ** 0.5         # conv fan-in scale = 1/48
N_CORES = 8
BPC = B // N_CORES                      # samples per core = 4
SPP = SP + 2                            # padded image dim = 66
CKK = COUT * K * K                      # 2304 free columns in weight layout
YX = SP * SP                            # 4096 spatial positions

F32 = mybir.dt.float32
F32R = mybir.dt.float32r
AF = mybir.ActivationFunctionType
ALU = mybir.AluOpType


def _split_multi_waits(bir: dict) -> int:
    """Hoist all but one sync wait from every instruction onto single-wait
    EventSemaphore carriers inserted immediately before it (same engine)."""
    ctr = 0
    for fn in bir.get("functions", []):
        for blk in fn.get("blocks", []):
            insts = blk.get("instructions", [])
            if not any(
                len(((i.get("sync_info") or {}).get("on_wait") or [])) > 1
                for i in insts
            ):
                continue
            new_insts = []
            for inst in insts:
                si = inst.get("sync_info")
                ow = (si or {}).get("on_wait") or []
                if len(ow) > 1:
                    for w in ow[:-1]:
                        ctr += 1
                        new_insts.append({
                            "debug": inst.get("debug", 0),
                            "engine": inst["engine"],
                            "ins": [],
                            "outs": [],
                            "name": f"waitsplit-{ctr}",
                            "opcode": "EventSemaphore",
                            "sync_info": {"on_update": [], "on_wait": [w]},
                        })
                    si["on_wait"] = [ow[-1]]
                new_insts.append(inst)
            blk["instructions"] = new_insts
    return ctr


class _WaitSplitBass(bass.Bass):
    def to_json_bytes(self) -> bytes:
        raw = super().to_json_bytes()
        bir = json.loads(raw)
        if _split_multi_waits(bir):
            return json.dumps(bir).encode()
        return raw


def _pbcast(ap, n):
    """Manual 0-step partition broadcast AP (DMA-only; engines reject it)."""
    return bass.AP(tensor=ap.tensor, offset=ap.offset,
                   ap=[[0, n]] + [list(d) for d in ap.ap[1:]])


def _build_program() -> bass.Bass:
    nc = _WaitSplitBass("TRN2", target_bir_lowering=False, debug=False)

    x_d = nc.dram_tensor("x", [BPC, CIN, SPP, SPP], F32, kind="ExternalInput")
    styleT_d = nc.dram_tensor("styleT", [STYLE_DIM, BPC], F32, kind="ExternalInput")
    wT_d = nc.dram_tensor("wT", [CIN, CKK], F32, kind="ExternalInput")
    mod_wT_d = nc.dram_tensor("mod_wT", [256, CIN], F32, kind="ExternalInput")
    mod_b_d = nc.dram_tensor("mod_b", [CIN, 1], F32, kind="ExternalInput")
    sp_wT_d = nc.dram_tensor("sp_wT", [256, YX], F32, kind="ExternalInput")
    sp_b_d = nc.dram_tensor("sp_b", [1, YX], F32, kind="ExternalInput")
    out_d = nc.dram_tensor("out", [BPC, COUT, SP, SP], F32, kind="ExternalOutput")
    spm_d = nc.dram_tensor("spm_scratch", [BPC, YX], F32, kind="Internal")
    dspt_d = nc.dram_tensor("dspt_scratch", [BPC, 1], F32, kind="Internal")

    with tile.TileContext(nc) as tc:
        with tc.tile_pool(name="const", bufs=1) as cpool, \
             tc.tile_pool(name="persist", bufs=1) as ppool, \
             tc.tile_pool(name="sps", bufs=1, space="PSUM") as spsum:

            # ---------- constants ----------
            onesF = cpool.tile([128, 2], F32, name="onesF")
            nc.vector.memset(onesF, 1.0)
            ones = cpool.tile([128, 2], F32R, name="ones")
            nc.vector.tensor_copy(ones, onesF)
            eps4 = cpool.tile([BPC, 1], F32, name="eps4")
            nc.vector.memset(eps4, EPS)
            eps128 = cpool.tile([128, 1], F32, name="eps128")
            nc.vector.memset(eps128, EPS)

            # ---------- persistent weights / style ----------
            wt = [ppool.tile([128, CKK], F32, name=f"wt{k}") for k in range(2)]
            stylec = [ppool.tile([128, BPC], F32, name=f"stylec{k}") for k in range(2)]
            spm = ppool.tile([BPC, YX], F32, name="spm")
            _xp_cm = tc.tile_pool(name="xp", bufs=2)
            xppool = _xp_cm.__enter__()
            _sw_cm = tc.tile_pool(name="swtc", bufs=8)
            swpool = _sw_cm.__enter__()

            def load_xp(xp, b):
                # interleave k0/k1 row-halves so both ci-tiles' early rows
                # land first and conv groups can start while the rest streams
                half = (SPP // 2) * SPP
                for h in range(2):
                    lo, hi = h * half, (h + 1) * half if h == 1 else half
                    hi = half if h == 0 else SPP * SPP
                    for k in range(2):
                        src = x_d.ap()[b, k * 128:(k + 1) * 128, :, :] \
                            .rearrange("p r c -> p (r c)")[:, lo:hi].bitcast(F32R)
                        nc.sync.dma_start(out=xp[k][:, lo:hi], in_=src)

            # ---------- setup (pool freed afterwards) ----------
            with tc.tile_pool(name="setup", bufs=1) as spool, \
                 tc.tile_pool(name="setup_ps", bufs=1, space="PSUM") as supsum:
                mw = [spool.tile([128, CIN], F32, name=f"mw{k}") for k in range(2)]
                stc = [spool.tile([128, BPC], F32, name=f"stc{k}") for k in range(2)]
                sts = [ppool.tile([128, BPC], F32R, name=f"sts{k}") for k in range(2)]
                mb = [spool.tile([128, 1], F32, name=f"mb{k}") for k in range(2)]
                spb = ppool.tile([BPC, YX], F32, name="spb", tag="spsc")
                scratch = ppool.tile([BPC, YX], F32, name="scratch", tag="spsc")
                # small loads first (style path gates the first conv matmul);
                # gpsimd SWDGE comes up several us before the HWDGE queues
                for k in range(2):
                    nc.gpsimd.dma_start(out=mw[k], in_=mod_wT_d.ap()[k * 128:(k + 1) * 128, :])
                    nc.gpsimd.dma_start(out=stc[k], in_=styleT_d.ap()[k * 128:(k + 1) * 128, :])
                    nc.gpsimd.dma_start(
                        out=sts[k],
                        in_=styleT_d.ap()[256 + k * 128:256 + (k + 1) * 128, :].bitcast(F32R),
                    )
                    nc.gpsimd.dma_start(out=mb[k], in_=mod_b_d.ap()[k * 128:(k + 1) * 128, :])
                nc.gpsimd.dma_start(out=spb, in_=_pbcast(sp_b_d.ap(), BPC))
                # then the conv weights, then sample 0's image, then sp_wT
                for k in range(2):
                    nc.sync.dma_start(out=wt[k], in_=wT_d.ap()[k * 128:(k + 1) * 128, :])
                xp0 = [
                    xppool.tile([128, SPP * SPP], F32R, name=f"xp{k}_0", tag=f"xp{k}")
                    for k in range(2)
                ]
                _half = (SPP // 2) * SPP
                for k in range(2):
                    nc.sync.dma_start(
                        out=xp0[k][:, 0:_half],
                        in_=x_d.ap()[0, k * 128:(k + 1) * 128, :, :]
                            .rearrange("p r c -> p (r c)")[:, 0:_half].bitcast(F32R),
                    )

                # channel style: stylec[ci, b] = CS*(mod_w@chan*LS + mod_b)
                for m in range(2):
                    ps_style = supsum.tile([128, BPC], F32, name="ps_style", tag="ps_style")
                    for k in range(2):
                        nc.tensor.matmul(
                            ps_style, mw[k][:, m * 128:(m + 1) * 128], stc[k],
                            start=(k == 0), stop=(k == 1),
                        )
                    mbcs = spool.tile([128, 1], F32, name=f"mbcs{m}")
                    nc.scalar.mul(mbcs, mb[m], CS)
                    nc.scalar.activation(
                        out=stylec[m], in_=ps_style, func=AF.Identity,
                        bias=mbcs, scale=LS * CS,
                    )

                # spatial map: spm[b, yx] = sp_psum*LS + sp_b
                # Per-chunk pipeline: matmul -> (bias+scale, fused square
                # accumulation) -> immediate DRAM staging of the UNSCALED map.
                # The global spatial demod factor is folded into the per-co
                # demod column (dcol) later, so nothing here serializes on the
                # full map.
                sums = ppool.tile([BPC, 8], F32, name="sums")
                sp_mms = []
                for n in range(8):
                    ps_sp = spsum.tile([BPC, 512], F32, name="ps_sp", tag="ps_sp")
                    for k in range(2):
                        swtc = swpool.tile([128, 512], F32R,
                                           name=f"swtc_{n}_{k}", tag="swtc")
                        nc.sync.dma_start(
                            out=swtc,
                            in_=sp_wT_d.ap()[k * 128:(k + 1) * 128,
                                             n * 512:(n + 1) * 512].bitcast(F32R),
                        )
                        sp_mms.append(nc.tensor.matmul(
                            ps_sp, sts[k], swtc,
                            start=(k == 0), stop=(k == 1),
                        ))
                    nc.vector.scalar_tensor_tensor(
                        out=spm[:, n * 512:(n + 1) * 512], in0=ps_sp, scalar=LS,
                        in1=spb[:, n * 512:(n + 1) * 512],
                        op0=ALU.mult, op1=ALU.add,
                    )
                    nc.gpsimd.dma_start(
                        out=spm_d.ap()[:, n * 512:(n + 1) * 512],
                        in_=spm[:, n * 512:(n + 1) * 512],
                    )
                    nc.vector.scalar_tensor_tensor(
                        out=scratch[:, n * 512:(n + 1) * 512],
                        in0=spm[:, n * 512:(n + 1) * 512], scalar=1.0,
                        in1=spm[:, n * 512:(n + 1) * 512],
                        op0=ALU.mult, op1=ALU.mult,
                        accum_out=sums[:, n:n + 1],
                    )

                # global spatial demod scalar: dspt = sqrt(YX/sum + eps)
                ssq = ppool.tile([BPC, 1], F32, name="ssq")
                nc.vector.reduce_sum(out=ssq, in_=sums, axis=mybir.AxisListType.X)
                rsq = ppool.tile([BPC, 1], F32, name="rsq")
                nc.vector.reciprocal(rsq, ssq)
                dspt = ppool.tile([BPC, 1], F32, name="dspt")
                nc.scalar.activation(
                    out=dspt, in_=rsq, func=AF.Sqrt, bias=eps4, scale=float(YX),
                )
                nc.gpsimd.dma_start(out=dspt_d.ap(), in_=dspt)
                for k in range(2):
                    nc.sync.dma_start(
                        out=xp0[k][:, _half:SPP * SPP],
                        in_=x_d.ap()[0, k * 128:(k + 1) * 128, :, :]
                            .rearrange("p r c -> p (r c)")[:, _half:SPP * SPP]
                            .bitcast(F32R),
                    )

            # ---------- per-sample pipeline ----------
            _stack = ExitStack()
            cpsum = _stack.enter_context(tc.tile_pool(name="cps", bufs=6, space="PSUM"))
            wscpool = _stack.enter_context(tc.tile_pool(name="wsc", bufs=2))
            wsqpool = _stack.enter_context(tc.tile_pool(name="wsq", bufs=1))
            dempool = _stack.enter_context(tc.tile_pool(name="dem", bufs=2))
            opool = _stack.enter_context(tc.tile_pool(name="ot", bufs=3))
            smpool = _stack.enter_context(tc.tile_pool(name="smb", bufs=2))

            for b in range(BPC):
                # modulated (pre-demod) weight: wsc = wt * (CS*style[ci,b])
                wsc = [
                    wscpool.tile([128, CKK], F32R, name=f"wsc{k}_{b}", tag=f"wsc{k}")
                    for k in range(2)
                ]
                wsq = [
                    wsqpool.tile([128, CKK], F32, name=f"wsq{k}_{b}", tag="wsq")
                    for k in range(2)
                ]
                wsqk = [
                    dempool.tile([128, COUT], F32R, name=f"wsqk{k}_{b}", tag=f"wsqk{k}")
                    for k in range(2)
                ]
                for k in range(2):
                    nc.vector.tensor_scalar_mul(wsc[k], wt[k], stylec[k][:, b:b + 1])
                    nc.scalar.activation(out=wsq[k], in_=wsc[k].bitcast(F32),
                                         func=AF.Square)
                    with nc.allow_low_precision(reason="f32r is fp32-width"):
                        nc.vector.reduce_sum(
                            out=wsqk[k],
                            in_=wsq[k].rearrange("p (co kk) -> p co kk", kk=9),
                            axis=mybir.AxisListType.X,
                        )

                # per-sample spatial demod scalar, replicated to 128 partitions
                dsptb = dempool.tile([128, 1], F32, name=f"dsptb_{b}", tag="dsptb")
                nc.gpsimd.dma_start(out=dsptb, in_=_pbcast(dspt_d.ap()[b:b + 1, :], 128))

                # per-co demod, partition-native: ps_d[co,1] = sum_ci wsqk
                dcol = []
                for m in range(2):
                    ps_d = spsum.tile([128, 2], F32, name=f"ps_d_{b}_{m}", tag="ps_d")
                    for k in range(2):
                        nc.tensor.matmul(
                            ps_d, wsqk[k][:, m * 128:(m + 1) * 128], ones,
                            start=(k == 0), stop=(k == 1),
                        )
                    dsq = dempool.tile([128, 1], F32, name=f"dsq_{b}_{m}", tag=f"dsq{m}")
                    nc.scalar.activation(out=dsq, in_=ps_d[:, 0:1], func=AF.Sqrt,
                                         bias=eps128, scale=1.0)
                    dc = dempool.tile([128, 1], F32, name=f"dcol_{b}_{m}", tag=f"dcol{m}")
                    nc.vector.reciprocal(dc, dsq)
                    nc.vector.tensor_mul(dc, dc, dsptb)
                    dcol.append(dc)

                # padded input image [128ci, 66, 66] per ci-tile
                if b == 0:
                    xp = xp0
                else:
                    xp = [
                        xppool.tile([128, SPP * SPP], F32R, name=f"xp{k}_{b}", tag=f"xp{k}")
                        for k in range(2)
                    ]
                    load_xp(xp, b)

                # conv + fused epilogue: out = (psum * demod[co]) * spmap[yx]
                for n in range(8):
                    smb = smpool.tile([128, 512], F32, name=f"smb_{b}_{n}", tag="smb")
                    nc.gpsimd.dma_start(
                        out=smb,
                        in_=_pbcast(spm_d.ap()[b:b + 1, n * 512:(n + 1) * 512], 128),
                    )
                    for m in range(2):
                        ps = cpsum.tile([128, 512], F32, name=f"ps_{b}_{m}_{n}", tag="ps")
                        i = 0
                        for k in range(2):
                            wv = wsc[k].rearrange("p (co kk) -> p co kk", kk=9)
                            xpv = xp[k].rearrange("p (r c) -> p r c", c=SPP)
                            for s in range(9):
                                dy, dx = s // 3, s % 3
                                mm = nc.tensor.matmul(
                                    ps,
                                    wv[:, m * 128:(m + 1) * 128, s],
                                    xpv[:, n * 8 + dy:n * 8 + dy + 8, dx:dx + SP],
                                    start=(i == 0), stop=(i == 17),
                                )
                                i += 1
                        if b == 0 and n == 0 and m == 0 and sp_mms:
                            # Keep the spatial-map matmuls out of the PE
                            # stream until sample-0 conv is well underway
                            # (their sp_wT input streams in slowly; scheduling
                            # them early head-of-line-blocks the PE).
                            for _sp in sp_mms:
                                add_dep_helper(
                                    _sp.ins, mm.ins, sync=False,
                                    reason="sp-map after early sample-0 conv",
                                )
                            sp_mms = []
                        ot = opool.tile([128, 512], F32, name=f"ot_{b}_{m}_{n}", tag="ot")
                        nc.vector.scalar_tensor_tensor(
                            out=ot, in0=ps, scalar=dcol[m][:, 0:1], in1=smb,
                            op0=ALU.mult, op1=ALU.mult,
                        )
                        nc.sync.dma_start(
                            out=out_d.ap()[b, m * 128:(m + 1) * 128, n * 8:(n + 1) * 8, :],
                            in_=ot.rearrange("p (r c) -> p r c", c=SP),
                        )
            _stack.close()
            _sw_cm.__exit__(None, None, None)
            _xp_cm.__exit__(None, None, None)
    return nc


_prog_cache = {}


def _get_program() -> bass.Bass:
    if "nc" not in _prog_cache:
        _prog_cache["nc"] = _build_program()
    return _prog_cache["nc"]


def _make_in_maps(inputs):
    x = np.asarray(inputs["x"], dtype=np.float32)
    x = np.pad(x, ((0, 0), (0, 0), (1, 1), (1, 1)))
    style_in = np.asarray(inputs["style_in"], dtype=np.float32)
    weight = np.asarray(inputs["weight"], dtype=np.float32)
    mod_w = np.asarray(inputs["mod_w"], dtype=np.float32)
    mod_b = np.asarray(inputs["mod_b"], dtype=np.float32)
    sp_w = np.asarray(inputs["sp_w"], dtype=np.float32)
    sp_b = np.asarray(inputs["sp_b"], dtype=np.float32)

    # replicated parameter layouts (pure transposes/reshapes, no arithmetic)
    wT = np.ascontiguousarray(
        weight[0].transpose(1, 0, 2, 3).reshape(CIN, CKK))       # [ci, co*9]
    mod_wT = np.ascontiguousarray(mod_w.T)                        # [sd, ci]
    mod_b2 = np.ascontiguousarray(mod_b.reshape(CIN, 1))
    sp_wT = np.ascontiguousarray(sp_w.T)                          # [sd, yx]
    sp_b2 = np.ascontiguousarray(sp_b.reshape(1, YX))

    in_maps = []
    for c in range(N_CORES):
        sl = slice(c * BPC, (c + 1) * BPC)
        in_maps.append({
            "x": np.ascontiguousarray(x[sl]),
            "styleT": np.ascontiguousarray(style_in[sl].T),
            "wT": wT,
            "mod_wT": mod_wT,
            "mod_b": mod_b2,
            "sp_wT": sp_wT,
            "sp_b": sp_b2,
        })
    return in_maps


def _run(inputs, trace=False):
    nc = _get_program()
    in_maps = _make_in_maps(inputs)
    res = run_bass_kernel_spmd(nc, in_maps, core_ids=list(range(N_CORES)), trace=trace)
    out = np.concatenate([res.results[c]["out"] for c in range(N_CORES)], axis=0)
    return out, res


def kernel(**inputs) -> np.ndarray:
    out, _ = _run(inputs, trace=False)
    return out



# revision 13
# speedup vs baseline: 1.1259x; 1.1259x over previous
"""Trainium2 Bass kernel for nn_ChannelSpatialModulatedConv2d.

Data-parallel over batch across 8 NeuronCores (4 samples each). Per core:
  1. style  = style_chan @ (mod_w*ls).T + mod_b             (PE, fp32)
  2. wsc    = conv_scale * weight * style[b,ci]             (DVE tensor_scalar)
     demod  = rsqrt(sum(wsc^2) over (ci,kk) + eps) per co   (ACT square, DVE
              kk-reduce, PE ones-matmul -> [128co,1] partition-native)
  3. conv2d(x[b], wsc) via 18 accumulating fp32r matmuls per [128co x 512yx]
     PSUM tile (2 ci-tiles x 9 shifts), shifted-window APs over a zero-padded
     66x66 SBUF image.
  4. sp map = style_sp @ (sp_w*ls).T + sp_b, spatially demodulated. The PSUM
     epilogue computes out = (psum * demod[co]) * spmap[yx] in one fused
     scalar_tensor_tensor (demod is NOT folded into the weights; conv is
     linear so this is equivalent).

The baked walrus build only supports ONE sync wait per instruction, so the
Bass subclass rewrites the scheduled BIR JSON, hoisting extra waits onto
single-wait EventSemaphore carriers inserted before the instruction (same
engine => identical blocking semantics).
"""

import json
import sys
from contextlib import ExitStack

for _p in ("/opt/pypackages", "/opt/trn_rl_repo"):
    if _p not in sys.path:
        sys.path.insert(0, _p)

import ml_dtypes
import numpy as np

import concourse.bass as bass
import concourse.mybir as mybir
import concourse.tile as tile
from concourse.tile_rust import add_dep_helper
from concourse.bass_utils import run_bass_kernel_spmd

# Problem constants (hardcoded per harness contract)
B, CIN, COUT, K = 32, 256, 256, 3
STYLE_DIM, SP = 512, 64
EPS = 1e-6
LS = 1.0 / (STYLE_DIM // 2) ** 0.5      # EqualLinear scale = 1/16
CS = 1.0 / (CIN * K * K) ** 0.5         # conv fan-in scale = 1/48
N_CORES = 8
BPC = B // N_CORES                      # samples per core = 4
SPP = SP + 2                            # padded image dim = 66
CKK = COUT * K * K                      # 2304 free columns in weight layout
YX = SP * SP                            # 4096 spatial positions

F32 = mybir.dt.float32
F32R = mybir.dt.float32r
BF16 = mybir.dt.bfloat16
AF = mybir.ActivationFunctionType
ALU = mybir.AluOpType


def _split_multi_waits(bir: dict) -> int:
    """Hoist all but one sync wait from every instruction onto single-wait
    EventSemaphore carriers inserted immediately before it (same engine)."""
    ctr = 0
    for fn in bir.get("functions", []):
        for blk in fn.get("blocks", []):
            insts = blk.get("instructions", [])
            if not any(
                len(((i.get("sync_info") or {}).get("on_wait") or [])) > 1
                for i in insts
            ):
                continue
            new_insts = []
            for inst in insts:
                si = inst.get("sync_info")
                ow = (si or {}).get("on_wait") or []
                if len(ow) > 1:
                    for w in ow[:-1]:
                        ctr += 1
                        new_insts.append({
                            "debug": inst.get("debug", 0),
                            "engine": inst["engine"],
                            "ins": [],
                            "outs": [],
                            "name": f"waitsplit-{ctr}",
                            "opcode": "EventSemaphore",
                            "sync_info": {"on_update": [], "on_wait": [w]},
                        })
                    si["on_wait"] = [ow[-1]]
                new_insts.append(inst)
            blk["instructions"] = new_insts
    return ctr


class _WaitSplitBass(bass.Bass):
    def to_json_bytes(self) -> bytes:
        raw = super().to_json_bytes()
        bir = json.loads(raw)
        if _split_multi_waits(bir):
            return json.dumps(bir).encode()
        return raw


def _pbcast(ap, n):
    """Manual 0-step partition broadcast AP (DMA-only; engines reject it)."""
    return bass.AP(tensor=ap.tensor, offset=ap.offset,
                   ap=[[0, n]] + [list(d) for d in ap.ap[1:]])


def _build_program() -> bass.Bass:
    nc = _WaitSplitBass("TRN2", target_bir_lowering=False, debug=False)

    x_d = nc.dram_tensor("x", [BPC, CIN, SPP, SPP], BF16, kind="ExternalInput")
    styleT_d = nc.dram_tensor("styleT", [STYLE_DIM, BPC], F32, kind="ExternalInput")
    wT_d = nc.dram_tensor("wT", [CIN, CKK], BF16, kind="ExternalInput")
    mod_wT_d = nc.dram_tensor("mod_wT", [256, CIN], F32, kind="ExternalInput")
    mod_b_d = nc.dram_tensor("mod_b", [CIN, 1], F32, kind="ExternalInput")
    sp_wT_d = nc.dram_tensor("sp_wT", [256, YX], F32, kind="ExternalInput")
    sp_b_d = nc.dram_tensor("sp_b", [1, YX], F32, kind="ExternalInput")
    out_d = nc.dram_tensor("out", [BPC, COUT, SP, SP], F32, kind="ExternalOutput")
    spm_d = nc.dram_tensor("spm_scratch", [BPC, YX], F32, kind="Internal")
    dspt_d = nc.dram_tensor("dspt_scratch", [BPC, 1], F32, kind="Internal")

    with tile.TileContext(nc) as tc:
        with tc.tile_pool(name="const", bufs=1) as cpool, \
             tc.tile_pool(name="persist", bufs=1) as ppool, \
             tc.tile_pool(name="sps", bufs=1, space="PSUM") as spsum:

            # ---------- constants ----------
            onesF = cpool.tile([128, 2], F32, name="onesF")
            nc.vector.memset(onesF, 1.0)
            ones = cpool.tile([128, 2], F32R, name="ones")
            nc.vector.tensor_copy(ones, onesF)
            eps4 = cpool.tile([BPC, 1], F32, name="eps4")
            nc.vector.memset(eps4, EPS)
            eps128 = cpool.tile([128, 1], F32, name="eps128")
            nc.vector.memset(eps128, EPS)

            # ---------- persistent weights / style ----------
            wt = [ppool.tile([128, CKK], BF16, name=f"wt{k}") for k in range(2)]
            stylec = [ppool.tile([128, BPC], F32, name=f"stylec{k}") for k in range(2)]
            spm = ppool.tile([BPC, YX], F32, name="spm")
            _xp_cm = tc.tile_pool(name="xp", bufs=2)
            xppool = _xp_cm.__enter__()
            _sw_cm = tc.tile_pool(name="swtc", bufs=8)
            swpool = _sw_cm.__enter__()

            def load_xp(xp, b):
                # interleave k0/k1 row-halves so both ci-tiles' early rows
                # land first and conv groups can start while the rest streams
                half = (SPP // 2) * SPP
                for h in range(2):
                    lo, hi = h * half, (h + 1) * half if h == 1 else half
                    hi = half if h == 0 else SPP * SPP
                    for k in range(2):
                        src = x_d.ap()[b, k * 128:(k + 1) * 128, :, :] \
                            .rearrange("p r c -> p (r c)")[:, lo:hi]
                        nc.sync.dma_start(out=xp[k][:, lo:hi], in_=src)

            # ---------- setup (pool freed afterwards) ----------
            with tc.tile_pool(name="setup", bufs=1) as spool, \
                 tc.tile_pool(name="setup_ps", bufs=1, space="PSUM") as supsum:
                mw = [spool.tile([128, CIN], F32, name=f"mw{k}") for k in range(2)]
                stc = [spool.tile([128, BPC], F32, name=f"stc{k}") for k in range(2)]
                sts = [ppool.tile([128, BPC], F32R, name=f"sts{k}") for k in range(2)]
                mb = [spool.tile([128, 1], F32, name=f"mb{k}") for k in range(2)]
                spb = ppool.tile([BPC, YX], F32, name="spb", tag="spsc")
                scratch = ppool.tile([BPC, YX], F32, name="scratch", tag="spsc")
                # small loads first (style path gates the first conv matmul);
                # gpsimd SWDGE comes up several us before the HWDGE queues
                for k in range(2):
                    nc.gpsimd.dma_start(out=mw[k], in_=mod_wT_d.ap()[k * 128:(k + 1) * 128, :])
                    nc.gpsimd.dma_start(out=stc[k], in_=styleT_d.ap()[k * 128:(k + 1) * 128, :])
                    nc.gpsimd.dma_start(
                        out=sts[k],
                        in_=styleT_d.ap()[256 + k * 128:256 + (k + 1) * 128, :].bitcast(F32R),
                    )
                    nc.gpsimd.dma_start(out=mb[k], in_=mod_b_d.ap()[k * 128:(k + 1) * 128, :])
                nc.gpsimd.dma_start(out=spb, in_=_pbcast(sp_b_d.ap(), BPC))
                # then the conv weights, then sample 0's image, then sp_wT
                for k in range(2):
                    nc.sync.dma_start(out=wt[k], in_=wT_d.ap()[k * 128:(k + 1) * 128, :])
                xp0 = [
                    xppool.tile([128, SPP * SPP], BF16, name=f"xp{k}_0", tag=f"xp{k}")
                    for k in range(2)
                ]
                _half = (SPP // 2) * SPP
                for k in range(2):
                    nc.sync.dma_start(
                        out=xp0[k][:, 0:_half],
                        in_=x_d.ap()[0, k * 128:(k + 1) * 128, :, :]
                            .rearrange("p r c -> p (r c)")[:, 0:_half],
                    )

                # channel style: stylec[ci, b] = CS*(mod_w@chan*LS + mod_b)
                for m in range(2):
                    ps_style = supsum.tile([128, BPC], F32, name="ps_style", tag="ps_style")
                    for k in range(2):
                        nc.tensor.matmul(
                            ps_style, mw[k][:, m * 128:(m + 1) * 128], stc[k],
                            start=(k == 0), stop=(k == 1),
                        )
                    mbcs = spool.tile([128, 1], F32, name=f"mbcs{m}")
                    nc.scalar.mul(mbcs, mb[m], CS)
                    nc.scalar.activation(
                        out=stylec[m], in_=ps_style, func=AF.Identity,
                        bias=mbcs, scale=LS * CS,
                    )

                # spatial map: spm[b, yx] = sp_psum*LS + sp_b
                # Per-chunk pipeline: matmul -> (bias+scale, fused square
                # accumulation) -> immediate DRAM staging of the UNSCALED map.
                # The global spatial demod factor is folded into the per-co
                # demod column (dcol) later, so nothing here serializes on the
                # full map.
                sums = ppool.tile([BPC, 8], F32, name="sums")
                sp_mms = []
                for n in range(8):
                    ps_sp = spsum.tile([BPC, 512], F32, name="ps_sp", tag="ps_sp")
                    for k in range(2):
                        swtc = swpool.tile([128, 512], F32R,
                                           name=f"swtc_{n}_{k}", tag="swtc")
                        nc.sync.dma_start(
                            out=swtc,
                            in_=sp_wT_d.ap()[k * 128:(k + 1) * 128,
                                             n * 512:(n + 1) * 512].bitcast(F32R),
                        )
                        sp_mms.append(nc.tensor.matmul(
                            ps_sp, sts[k], swtc,
                            start=(k == 0), stop=(k == 1),
                        ))
                    nc.vector.scalar_tensor_tensor(
                        out=spm[:, n * 512:(n + 1) * 512], in0=ps_sp, scalar=LS,
                        in1=spb[:, n * 512:(n + 1) * 512],
                        op0=ALU.mult, op1=ALU.add,
                    )
                    nc.gpsimd.dma_start(
                        out=spm_d.ap()[:, n * 512:(n + 1) * 512],
                        in_=spm[:, n * 512:(n + 1) * 512],
                    )
                    nc.vector.scalar_tensor_tensor(
                        out=scratch[:, n * 512:(n + 1) * 512],
                        in0=spm[:, n * 512:(n + 1) * 512], scalar=1.0,
                        in1=spm[:, n * 512:(n + 1) * 512],
                        op0=ALU.mult, op1=ALU.mult,
                        accum_out=sums[:, n:n + 1],
                    )

                # global spatial demod scalar: dspt = sqrt(YX/sum + eps)
                ssq = ppool.tile([BPC, 1], F32, name="ssq")
                nc.vector.reduce_sum(out=ssq, in_=sums, axis=mybir.AxisListType.X)
                rsq = ppool.tile([BPC, 1], F32, name="rsq")
                nc.vector.reciprocal(rsq, ssq)
                dspt = ppool.tile([BPC, 1], F32, name="dspt")
                nc.scalar.activation(
                    out=dspt, in_=rsq, func=AF.Sqrt, bias=eps4, scale=float(YX),
                )
                nc.gpsimd.dma_start(out=dspt_d.ap(), in_=dspt)
                for k in range(2):
                    nc.sync.dma_start(
                        out=xp0[k][:, _half:SPP * SPP],
                        in_=x_d.ap()[0, k * 128:(k + 1) * 128, :, :]
                            .rearrange("p r c -> p (r c)")[:, _half:SPP * SPP],
                    )

            # ---------- per-sample pipeline ----------
            _stack = ExitStack()
            cpsum = _stack.enter_context(tc.tile_pool(name="cps", bufs=6, space="PSUM"))
            wscpool = _stack.enter_context(tc.tile_pool(name="wsc", bufs=2))
            wsqpool = _stack.enter_context(tc.tile_pool(name="wsq", bufs=1))
            dempool = _stack.enter_context(tc.tile_pool(name="dem", bufs=2))
            opool = _stack.enter_context(tc.tile_pool(name="ot", bufs=3))
            smpool = _stack.enter_context(tc.tile_pool(name="smb", bufs=2))

            for b in range(BPC):
                # modulated (pre-demod) weight: wsc = wt * (CS*style[ci,b])
                wsc = [
                    wscpool.tile([128, CKK], BF16, name=f"wsc{k}_{b}", tag=f"wsc{k}")
                    for k in range(2)
                ]
                wsq = [
                    wsqpool.tile([128, CKK], F32, name=f"wsq{k}_{b}", tag="wsq")
                    for k in range(2)
                ]
                wsqk = [
                    dempool.tile([128, COUT], F32R, name=f"wsqk{k}_{b}", tag=f"wsqk{k}")
                    for k in range(2)
                ]
                for k in range(2):
                    nc.vector.tensor_scalar_mul(wsc[k], wt[k], stylec[k][:, b:b + 1])
                    nc.scalar.activation(out=wsq[k], in_=wsc[k],
                                         func=AF.Square)
                    with nc.allow_low_precision(reason="f32r is fp32-width"):
                        nc.vector.reduce_sum(
                            out=wsqk[k],
                            in_=wsq[k].rearrange("p (co kk) -> p co kk", kk=9),
                            axis=mybir.AxisListType.X,
                        )

                # per-sample spatial demod scalar, replicated to 128 partitions
                dsptb = dempool.tile([128, 1], F32, name=f"dsptb_{b}", tag="dsptb")
                nc.gpsimd.dma_start(out=dsptb, in_=_pbcast(dspt_d.ap()[b:b + 1, :], 128))

                # per-co demod, partition-native: ps_d[co,1] = sum_ci wsqk
                dcol = []
                for m in range(2):
                    ps_d = spsum.tile([128, 2], F32, name=f"ps_d_{b}_{m}", tag="ps_d")
                    for k in range(2):
                        nc.tensor.matmul(
                            ps_d, wsqk[k][:, m * 128:(m + 1) * 128], ones,
                            start=(k == 0), stop=(k == 1),
                        )
                    dsq = dempool.tile([128, 1], F32, name=f"dsq_{b}_{m}", tag=f"dsq{m}")
                    nc.scalar.activation(out=dsq, in_=ps_d[:, 0:1], func=AF.Sqrt,
                                         bias=eps128, scale=1.0)
                    dc = dempool.tile([128, 1], F32, name=f"dcol_{b}_{m}", tag=f"dcol{m}")
                    nc.vector.reciprocal(dc, dsq)
                    nc.vector.tensor_mul(dc, dc, dsptb)
                    dcol.append(dc)

                # padded input image [128ci, 66, 66] per ci-tile
                if b == 0:
                    xp = xp0
                else:
                    xp = [
                        xppool.tile([128, SPP * SPP], BF16, name=f"xp{k}_{b}", tag=f"xp{k}")
                        for k in range(2)
                    ]
                    load_xp(xp, b)

                # conv + fused epilogue: out = (psum * demod[co]) * spmap[yx]
                for n in range(8):
                    smb = smpool.tile([128, 512], F32, name=f"smb_{b}_{n}", tag="smb")
                    nc.gpsimd.dma_start(
                        out=smb,
                        in_=_pbcast(spm_d.ap()[b:b + 1, n * 512:(n + 1) * 512], 128),
                    )
                    for m in range(2):
                        ps = cpsum.tile([128, 512], F32, name=f"ps_{b}_{m}_{n}", tag="ps")
                        i = 0
                        for k in range(2):
                            wv = wsc[k].rearrange("p (co kk) -> p co kk", kk=9)
                            xpv = xp[k].rearrange("p (r c) -> p r c", c=SPP)
                            for s in range(9):
                                dy, dx = s // 3, s % 3
                                mm = nc.tensor.matmul(
                                    ps,
                                    wv[:, m * 128:(m + 1) * 128, s],
                                    xpv[:, n * 8 + dy:n * 8 + dy + 8, dx:dx + SP],
                                    start=(i == 0), stop=(i == 17),
                                )
                                i += 1
                        if b == 0 and n == 0 and m == 0 and sp_mms:
                            # Keep the spatial-map matmuls out of the PE
                            # stream until sample-0 conv is well underway
                            # (their sp_wT input streams in slowly; scheduling
                            # them early head-of-line-blocks the PE).
                            for _sp in sp_mms:
                                add_dep_helper(
                                    _sp.ins, mm.ins, sync=False,
                                    reason="sp-map after early sample-0 conv",
                                )
                            sp_mms = []
                        ot = opool.tile([128, 512], F32, name=f"ot_{b}_{m}_{n}", tag="ot")
                        nc.vector.scalar_tensor_tensor(
                            out=ot, in0=ps, scalar=dcol[m][:, 0:1], in1=smb,
                            op0=ALU.mult, op1=ALU.mult,
                        )
                        nc.sync.dma_start(
                            out=out_d.ap()[b, m * 128:(m + 1) * 128, n * 8:(n + 1) * 8, :],
                            in_=ot.rearrange("p (r c) -> p r c", c=SP),
                        )
            _stack.close()
            _sw_cm.__exit__(None, None, None)
            _xp_cm.__exit__(None, None, None)
    return nc


_prog_cache = {}


def _get_program() -> bass.Bass:
    if "nc" not in _prog_cache:
        _prog_cache["nc"] = _build_program()
    return _prog_cache["nc"]


def _make_in_maps(inputs):
    x = np.asarray(inputs["x"], dtype=np.float32)
    x = np.pad(x, ((0, 0), (0, 0), (1, 1), (1, 1))).astype(ml_dtypes.bfloat16)
    style_in = np.asarray(inputs["style_in"], dtype=np.float32)
    weight = np.asarray(inputs["weight"], dtype=np.float32)
    mod_w = np.asarray(inputs["mod_w"], dtype=np.float32)
    mod_b = np.asarray(inputs["mod_b"], dtype=np.float32)
    sp_w = np.asarray(inputs["sp_w"], dtype=np.float32)
    sp_b = np.asarray(inputs["sp_b"], dtype=np.float32)

    # replicated parameter layouts (pure transposes/reshapes, no arithmetic)
    wT = np.ascontiguousarray(
        weight[0].transpose(1, 0, 2, 3).reshape(CIN, CKK)).astype(ml_dtypes.bfloat16)  # [ci, co*9]
    mod_wT = np.ascontiguousarray(mod_w.T)                        # [sd, ci]
    mod_b2 = np.ascontiguousarray(mod_b.reshape(CIN, 1))
    sp_wT = np.ascontiguousarray(sp_w.T)                          # [sd, yx]
    sp_b2 = np.ascontiguousarray(sp_b.reshape(1, YX))

    in_maps = []
    for c in range(N_CORES):
        sl = slice(c * BPC, (c + 1) * BPC)
        in_maps.append({
            "x": np.ascontiguousarray(x[sl]),
            "styleT": np.ascontiguousarray(style_in[sl].T),
            "wT": wT,
            "mod_wT": mod_wT,
            "mod_b": mod_b2,
            "sp_wT": sp_wT,
            "sp_b": sp_b2,
        })
    return in_maps


def _run(inputs, trace=False):
    nc = _get_program()
    in_maps = _make_in_maps(inputs)
    res = run_bass_kernel_spmd(nc, in_maps, core_ids=list(range(N_CORES)), trace=trace)
    out = np.concatenate([res.results[c]["out"] for c in range(N_CORES)], axis=0)
    return out, res


def kernel(**inputs) -> np.ndarray:
    out, _ = _run(inputs, trace=False)
    return out



# revision 24
# speedup vs baseline: 1.1548x; 1.0257x over previous
"""Trainium2 Bass kernel for nn_ChannelSpatialModulatedConv2d.

Data-parallel over batch across 8 NeuronCores (4 samples each). Per core:
  1. style  = style_chan @ (mod_w*ls).T + mod_b            (PE fp32, packed DMA)
  2. demod for ALL (co, b) once: S2[ci,co] = sum_kk wt^2 (ACT square + DVE
     kk-reduce), demodsq[co,b] = S2.T @ stylec^2 (PE), dcol = rsqrt * dspt.
  3. conv2d(x[b], wsc[b]) in bf16: wsc = wt_bf16 * stylec[ci,b] (DVE),
     18 accumulating bf16 matmuls per [128co x 512yx] PSUM tile (2 ci-tiles
     x 9 shifts), shifted-window APs over a zero-padded 66x66 bf16 image
     loaded in row bands so the PE starts early.
  4. sp map (bf16 matmuls) = style_sp @ (sp_w*ls).T + sp_b, staged to DRAM
     and broadcast back per chunk (bf16). Epilogue per chunk:
     out = (psum * dcol[co,b]) * spmap[yx] in one fused DVE op.

DMA queues: sync(SP-HWDGE) = wt + x bands + outs; scalar(ACT-HWDGE) =
sp_wT chunks + spmap broadcasts + dspt broadcast + sp_b; gpsimd(SWDGE) =
small style pack + mod_b + spmap staging + dspt store.

The baked walrus build only supports ONE sync wait per instruction, so the
Bass subclass rewrites the scheduled BIR JSON, hoisting extra waits onto
single-wait EventSemaphore carriers inserted before the instruction (same
engine => identical blocking semantics).
"""

import json
import sys
from contextlib import ExitStack

for _p in ("/opt/pypackages", "/opt/trn_rl_repo"):
    if _p not in sys.path:
        sys.path.insert(0, _p)

import ml_dtypes
import numpy as np

import concourse.bass as bass
import concourse.mybir as mybir
import concourse.tile as tile
from concourse.bass_utils import run_bass_kernel_spmd

# Problem constants (hardcoded per harness contract)
B, CIN, COUT, K = 32, 256, 256, 3
STYLE_DIM, SP = 512, 64
EPS = 1e-6
LS = 1.0 / (STYLE_DIM // 2) ** 0.5      # EqualLinear scale = 1/16
CS = 1.0 / (CIN * K * K) ** 0.5         # conv fan-in scale = 1/48
N_CORES = 8
BPC = B // N_CORES                      # samples per core = 4
SPP = SP + 2                            # padded image dim = 66
CKK = COUT * K * K                      # 2304 free columns in weight layout
YX = SP * SP                            # 4096 spatial positions

F32 = mybir.dt.float32
BF16 = mybir.dt.bfloat16
AF = mybir.ActivationFunctionType
ALU = mybir.AluOpType

# x row bands per ci-tile: chunk n reads padded rows [8n, 8n+10)
XBANDS = [(0, 18), (18, 42), (42, 66)]


def _split_multi_waits(bir: dict) -> int:
    """Hoist all but one sync wait from every instruction onto single-wait
    EventSemaphore carriers inserted immediately before it (same engine)."""
    ctr = 0
    for fn in bir.get("functions", []):
        for blk in fn.get("blocks", []):
            insts = blk.get("instructions", [])
            if not any(
                len(((i.get("sync_info") or {}).get("on_wait") or [])) > 1
                for i in insts
            ):
                continue
            new_insts = []
            for inst in insts:
                si = inst.get("sync_info")
                ow = (si or {}).get("on_wait") or []
                if len(ow) > 1:
                    for w in ow[:-1]:
                        ctr += 1
                        new_insts.append({
                            "debug": inst.get("debug", 0),
                            "engine": inst["engine"],
                            "ins": [],
                            "outs": [],
                            "name": f"waitsplit-{ctr}",
                            "opcode": "EventSemaphore",
                            "sync_info": {"on_update": [], "on_wait": [w]},
                        })
                    si["on_wait"] = [ow[-1]]
                new_insts.append(inst)
            blk["instructions"] = new_insts
    return ctr


class _WaitSplitBass(bass.Bass):
    def to_json_bytes(self) -> bytes:
        raw = super().to_json_bytes()
        bir = json.loads(raw)
        if _split_multi_waits(bir):
            return json.dumps(bir).encode()
        return raw


def _pbcast(ap, n):
    """Manual 0-step partition broadcast AP (DMA-only; engines reject it)."""
    return bass.AP(tensor=ap.tensor, offset=ap.offset,
                   ap=[[0, n]] + [list(d) for d in ap.ap[1:]])


def _build_program() -> bass.Bass:
    nc = _WaitSplitBass("TRN2", target_bir_lowering=False, debug=False)

    x_d = nc.dram_tensor("x", [BPC, CIN, SPP, SPP], BF16, kind="ExternalInput")
    stylepk_d = nc.dram_tensor("stylepk", [256, 264], F32, kind="ExternalInput")
    wT_d = nc.dram_tensor("wT", [CIN, CKK], BF16, kind="ExternalInput")
    mod_b_d = nc.dram_tensor("mod_b", [CIN, 1], F32, kind="ExternalInput")
    sp_wT_d = nc.dram_tensor("sp_wT", [256, YX], BF16, kind="ExternalInput")
    sp_b_d = nc.dram_tensor("sp_b", [1, YX], F32, kind="ExternalInput")
    out_d = nc.dram_tensor("out", [BPC, COUT, SP, SP], F32, kind="ExternalOutput")
    spm_d = nc.dram_tensor("spm_scratch", [BPC, YX], BF16, kind="Internal")
    dspt_d = nc.dram_tensor("dspt_scratch", [BPC, 1], F32, kind="Internal")

    with tile.TileContext(nc) as tc:
        with tc.tile_pool(name="const", bufs=1) as cpool, \
             tc.tile_pool(name="persist", bufs=1) as ppool, \
             tc.tile_pool(name="sps", bufs=1, space="PSUM") as spsum:

            # ---------- constants ----------
            eps4 = cpool.tile([BPC, 1], F32, name="eps4")
            nc.vector.memset(eps4, EPS)
            eps128 = cpool.tile([128, 1], F32, name="eps128")
            nc.vector.memset(eps128, EPS)

            # ---------- persistent tiles ----------
            wt = [ppool.tile([128, CKK], BF16, name=f"wt{k}") for k in range(2)]
            stylec = [ppool.tile([128, BPC], F32, name=f"stylec{k}") for k in range(2)]
            sts = [ppool.tile([128, BPC], BF16, name=f"sts{k}") for k in range(2)]
            wsqk = [ppool.tile([128, COUT], F32, name=f"wsqk{k}") for k in range(2)]
            dcol_all = [ppool.tile([128, BPC], F32, name=f"dcol{m}") for m in range(2)]
            spm = ppool.tile([BPC, YX], BF16, name="spm")
            spb = ppool.tile([BPC, YX], F32, name="spb")
            sums = ppool.tile([BPC, 8], F32, name="sums")
            dsptb = ppool.tile([128, BPC], F32, name="dsptb")
            stp = [ppool.tile([128, 264], F32, name=f"stp{k}") for k in range(2)]
            mb = [ppool.tile([128, 1], F32, name=f"mb{k}") for k in range(2)]
            mbcs = [ppool.tile([128, 1], F32, name=f"mbcs{k}") for k in range(2)]
            stylec2 = [ppool.tile([128, BPC], F32, name=f"stylec2_{k}") for k in range(2)]

            _stack = ExitStack()
            xppool = _stack.enter_context(tc.tile_pool(name="xp", bufs=2))
            swpool = _stack.enter_context(tc.tile_pool(name="swtc", bufs=1))
            wsqpool = _stack.enter_context(tc.tile_pool(name="wsq", bufs=1))
            scrpool = _stack.enter_context(tc.tile_pool(name="scr", bufs=2))
            cpsum = _stack.enter_context(tc.tile_pool(name="cps", bufs=6, space="PSUM"))
            wscpool = _stack.enter_context(tc.tile_pool(name="wsc", bufs=2))
            opool = _stack.enter_context(tc.tile_pool(name="ot", bufs=8))
            smpool = _stack.enter_context(tc.tile_pool(name="smb", bufs=16))

            # ---------- DMA issue: small style pack first (gpsimd) ----------
            for k in range(2):
                nc.gpsimd.dma_start(out=stp[k], in_=stylepk_d.ap()[k * 128:(k + 1) * 128, :])
            for k in range(2):
                nc.gpsimd.dma_start(out=mb[k], in_=mod_b_d.ap()[k * 128:(k + 1) * 128, :])

            # sync queue: wt0, x0 band0, wt1, x0 bands 1-2
            xp0 = [
                xppool.tile([128, SPP * SPP], BF16, name=f"xp{k}_0", tag=f"xp{k}")
                for k in range(2)
            ]

            def load_band(xp, b, bi, k):
                lo, hi = XBANDS[bi]
                nc.sync.dma_start(
                    out=xp[k][:, lo * SPP:hi * SPP],
                    in_=x_d.ap()[b, k * 128:(k + 1) * 128, lo:hi, :]
                        .rearrange("p r c -> p (r c)"),
                )

            nc.sync.dma_start(out=wt[0], in_=wT_d.ap()[0:128, :])
            load_band(xp0, 0, 0, 0)
            load_band(xp0, 0, 0, 1)
            nc.sync.dma_start(out=wt[1], in_=wT_d.ap()[128:256, :])
            for bi in (1, 2):
                load_band(xp0, 0, bi, 0)
                load_band(xp0, 0, bi, 1)

            # gpsimd queue: sp_wT chunks (bf16); scalar queue: sp_b broadcast
            swtc = []
            for n in range(8):
                pair = []
                for k in range(2):
                    t = swpool.tile([128, 512], BF16, name=f"swtc_{n}_{k}", tag=f"swtc{n}_{k}")
                    nc.gpsimd.dma_start(
                        out=t,
                        in_=sp_wT_d.ap()[k * 128:(k + 1) * 128, n * 512:(n + 1) * 512],
                    )
                    pair.append(t)
                swtc.append(pair)
            nc.scalar.dma_start(out=spb, in_=_pbcast(sp_b_d.ap(), BPC))

            # ---------- style path (PE fp32 + DVE epilogue) ----------
            for m in range(2):
                nc.vector.tensor_scalar_mul(mbcs[m], mb[m], CS)
            for m in range(2):
                ps_st = spsum.tile([128, BPC], F32, name=f"ps_st{m}", tag="ps_small")
                for k in range(2):
                    nc.tensor.matmul(
                        ps_st, stp[k][:, m * 128:(m + 1) * 128], stp[k][:, 256:260],
                        start=(k == 0), stop=(k == 1),
                    )
                nc.vector.tensor_scalar(
                    out=stylec[m], in0=ps_st, scalar1=LS * CS, scalar2=mbcs[m],
                    op0=ALU.mult, op1=ALU.add,
                )
                nc.scalar.activation(out=stylec2[m], in_=stylec[m], func=AF.Square)
            for k in range(2):
                nc.vector.tensor_copy(sts[k], stp[k][:, 260:264])

            # ---------- S2 = sum_kk wt^2 (for demod): ACT squares here, the
            # DVE kk-reduces are emitted inside the sample loop AFTER wsc(b0)
            # so DVE's in-order stream doesn't delay the first conv chunk.
            wsq = [wsqpool.tile([128, CKK], F32, name=f"wsq{k}", tag=f"wsq{k}")
                   for k in range(2)]
            for k in range(2):
                nc.scalar.activation(out=wsq[k], in_=wt[k], func=AF.Square)

            # ---------- helpers ----------
            def emit_sp_chunk(n):
                """spatial-map chunk n: 2 bf16 MMs -> STT -> stage + square-acc."""
                ps_sp = spsum.tile([BPC, 512], F32, name=f"ps_sp{n}", tag="ps_sp")
                for k in range(2):
                    nc.tensor.matmul(
                        ps_sp, sts[k], swtc[n][k],
                        start=(k == 0), stop=(k == 1),
                    )
                nc.vector.scalar_tensor_tensor(
                    out=spm[:, n * 512:(n + 1) * 512], in0=ps_sp, scalar=LS,
                    in1=spb[:, n * 512:(n + 1) * 512],
                    op0=ALU.mult, op1=ALU.add,
                )
                nc.gpsimd.dma_start(
                    out=spm_d.ap()[:, n * 512:(n + 1) * 512],
                    in_=spm[:, n * 512:(n + 1) * 512],
                )
                scr = scrpool.tile([BPC, 512], BF16, name=f"scr{n}", tag="scr")
                nc.vector.scalar_tensor_tensor(
                    out=scr, in0=spm[:, n * 512:(n + 1) * 512], scalar=1.0,
                    in1=spm[:, n * 512:(n + 1) * 512],
                    op0=ALU.mult, op1=ALU.mult,
                    accum_out=sums[:, n:n + 1],
                )

            def emit_dspt():
                """global spatial demod scalar -> DRAM -> 128-bcast."""
                ssq = ppool.tile([BPC, 1], F32, name="ssq")
                nc.vector.reduce_sum(out=ssq, in_=sums, axis=mybir.AxisListType.X)
                rsq = ppool.tile([BPC, 1], F32, name="rsq")
                nc.vector.reciprocal(rsq, ssq)
                dspt = ppool.tile([BPC, 1], F32, name="dspt")
                nc.scalar.activation(
                    out=dspt, in_=rsq, func=AF.Sqrt, bias=eps4, scale=float(YX),
                )
                nc.gpsimd.dma_start(out=dspt_d.ap(), in_=dspt)
                nc.scalar.dma_start(
                    out=dsptb,
                    in_=_pbcast(dspt_d.ap().rearrange("b one -> one b"), 128),
                )

            def emit_demod():
                """demodsq[co,b] = S2.T @ stylec^2; dcol = rsqrt(.)*dspt."""
                for m in range(2):
                    ps_d = spsum.tile([128, BPC], F32, name=f"ps_d{m}", tag="ps_small")
                    for k in range(2):
                        nc.tensor.matmul(
                            ps_d, wsqk[k][:, m * 128:(m + 1) * 128], stylec2[k],
                            start=(k == 0), stop=(k == 1),
                        )
                    dsq = ppool.tile([128, BPC], F32, name=f"dsq{m}")
                    nc.scalar.activation(out=dsq, in_=ps_d, func=AF.Sqrt,
                                         bias=eps128, scale=1.0)
                    dinv = ppool.tile([128, BPC], F32, name=f"dinv{m}")
                    nc.vector.reciprocal(dinv, dsq)
                    nc.vector.tensor_mul(dcol_all[m], dinv, dsptb)

            # ---------- per-sample conv pipeline ----------
            xp_tiles = [xp0] + [None] * (BPC - 1)

            def prefetch_xp(b):
                xp = [
                    xppool.tile([128, SPP * SPP], BF16, name=f"xp{k}_{b}", tag=f"xp{k}")
                    for k in range(2)
                ]
                xp_tiles[b] = xp
                return xp

            wsc_tiles = [None] * BPC

            def make_wsc(b):
                ws = [
                    wscpool.tile([128, CKK], BF16, name=f"wsc{k}_{b}", tag=f"wsc{k}")
                    for k in range(2)
                ]
                for k in range(2):
                    nc.vector.tensor_scalar_mul(ws[k], wt[k], stylec[k][:, b:b + 1])
                wsc_tiles[b] = ws

            for b in range(BPC):
                if b == 0:
                    make_wsc(0)
                    # S2 kk-reduces (DVE) for the demod matmuls
                    for k in range(2):
                        nc.vector.reduce_sum(
                            out=wsqk[k],
                            in_=wsq[k].rearrange("p (co kk) -> p co kk", kk=9),
                            axis=mybir.AxisListType.X,
                        )
                wsc = wsc_tiles[b]

                xp = xp_tiles[b]
                prefetched = False

                # spmap broadcasts (scalar queue). For b==0 they're emitted
                # lazily at epilogue time (their spm_d staging dep lands
                # mid-conv); for b>0 burst them at sample start.
                smb = [None] * 8

                def load_smb(n, b=b):
                    t = smpool.tile([128, 512], BF16, name=f"smb_{b}_{n}", tag="smb")
                    nc.scalar.dma_start(
                        out=t,
                        in_=_pbcast(spm_d.ap()[b:b + 1, n * 512:(n + 1) * 512], 128),
                    )
                    return t

                if b > 0:
                    for n in range(8):
                        smb[n] = load_smb(n)

                def emit_epilogue(m, n, ps, b=b, smb=smb):
                    if smb[n] is None:
                        smb[n] = load_smb(n)
                    ot = opool.tile([128, 512], F32, name=f"ot_{b}_{m}_{n}", tag="ot")
                    nc.vector.scalar_tensor_tensor(
                        out=ot, in0=ps, scalar=dcol_all[m][:, b:b + 1], in1=smb[n],
                        op0=ALU.mult, op1=ALU.mult,
                    )
                    nc.sync.dma_start(
                        out=out_d.ap()[b, m * 128:(m + 1) * 128, n * 8:(n + 1) * 8, :],
                        in_=ot.rearrange("p (r c) -> p r c", c=SP),
                    )

                pending = []  # (m, n, ps) epilogues held until dcol_all exists
                for m in range(2):
                    for n in range(8):
                        ps = cpsum.tile([128, 512], F32, name=f"ps_{b}_{m}_{n}", tag="ps")
                        i = 0
                        for k in range(2):
                            wv = wsc[k].rearrange("p (co kk) -> p co kk", kk=9)
                            xpv = xp[k].rearrange("p (r c) -> p r c", c=SPP)
                            for s in range(9):
                                dy, dx = s // 3, s % 3
                                nc.tensor.matmul(
                                    ps,
                                    wv[:, m * 128:(m + 1) * 128, s],
                                    xpv[:, n * 8 + dy:n * 8 + dy + 8, dx:dx + SP],
                                    start=(i == 0), stop=(i == 17),
                                )
                                i += 1
                        # interleave setup work into sample-0 m=0 PE stream
                        if b == 0 and m == 0:
                            if n < 4:
                                emit_sp_chunk(2 * n)
                                emit_sp_chunk(2 * n + 1)
                                if n == 3:
                                    emit_dspt()
                            elif n == 4:
                                emit_demod()
                        # prefetch next sample's modulated weights in m=0
                        # (keeps DVE's in-order stream from stalling the
                        # sample boundary behind this sample's epilogues)
                        if m == 0 and n == 6 and b + 1 < BPC:
                            make_wsc(b + 1)
                        # prefetch next sample's image in m=1
                        if m == 1 and 1 <= n <= 3 and b + 1 < BPC:
                            if not prefetched:
                                xpn = prefetch_xp(b + 1)
                                prefetched = True
                            load_band(xpn, b + 1, n - 1, 0)
                            load_band(xpn, b + 1, n - 1, 1)
                        # epilogue: for b0/m0 chunks <=4, dcol_all doesn't
                        # exist yet (DVE is in-order) — hold until demod done
                        if b == 0 and m == 0 and n < 4:
                            pending.append((m, n, ps))
                        else:
                            for pm, pn, pps in pending:
                                emit_epilogue(pm, pn, pps)
                            pending = []
                            emit_epilogue(m, n, ps)
            _stack.close()
    return nc


_prog_cache = {}


def _get_program() -> bass.Bass:
    if "nc" not in _prog_cache:
        _prog_cache["nc"] = _build_program()
    return _prog_cache["nc"]


def _make_in_maps(inputs):
    x = np.asarray(inputs["x"], dtype=np.float32)
    x = np.pad(x, ((0, 0), (0, 0), (1, 1), (1, 1))).astype(ml_dtypes.bfloat16)
    style_in = np.asarray(inputs["style_in"], dtype=np.float32)
    weight = np.asarray(inputs["weight"], dtype=np.float32)
    mod_w = np.asarray(inputs["mod_w"], dtype=np.float32)
    mod_b = np.asarray(inputs["mod_b"], dtype=np.float32)
    sp_w = np.asarray(inputs["sp_w"], dtype=np.float32)
    sp_b = np.asarray(inputs["sp_b"], dtype=np.float32)

    # replicated parameter layouts (pure transposes/reshapes + dtype casts)
    wT = np.ascontiguousarray(
        weight[0].transpose(1, 0, 2, 3).reshape(CIN, CKK)).astype(ml_dtypes.bfloat16)
    mod_wT = np.ascontiguousarray(mod_w.T)                        # [sd, ci]
    mod_b2 = np.ascontiguousarray(mod_b.reshape(CIN, 1))
    sp_wT = np.ascontiguousarray(sp_w.T).astype(ml_dtypes.bfloat16)  # [sd, yx]
    sp_b2 = np.ascontiguousarray(sp_b.reshape(1, YX))

    in_maps = []
    for c in range(N_CORES):
        sl = slice(c * BPC, (c + 1) * BPC)
        stylepk = np.concatenate(
            [mod_wT,
             np.ascontiguousarray(style_in[sl, :256].T),
             np.ascontiguousarray(style_in[sl, 256:].T)], axis=1)
        in_maps.append({
            "x": np.ascontiguousarray(x[sl]),
            "stylepk": np.ascontiguousarray(stylepk),
            "wT": wT,
            "mod_b": mod_b2,
            "sp_wT": sp_wT,
            "sp_b": sp_b2,
        })
    return in_maps


def _run(inputs, trace=False):
    nc = _get_program()
    in_maps = _make_in_maps(inputs)
    res = run_bass_kernel_spmd(nc, in_maps, core_ids=list(range(N_CORES)), trace=trace)
    out = np.concatenate([res.results[c]["out"] for c in range(N_CORES)], axis=0)
    return out, res


def kernel(**inputs) -> np.ndarray:
    out, _ = _run(inputs, trace=False)
    return out


# revision 35
# speedup vs baseline: 1.1575x; 1.0023x over previous
"""Trainium2 Bass kernel for nn_ChannelSpatialModulatedConv2d.

Data-parallel over batch across 8 NeuronCores (4 samples each). Per core:
  1. style  = style_chan @ (mod_w*ls).T + mod_b            (PE fp32, packed DMA)
  2. demod for ALL (co, b) once: S2[ci,co] = sum_kk wt^2 (ACT square + DVE
     kk-reduce), demodsq[co,b] = S2.T @ stylec^2 (PE), dcol = rsqrt * dspt.
  3. conv2d(x[b], wsc[b]) in bf16: wsc = wt_bf16 * stylec[ci,b] (DVE),
     18 accumulating bf16 matmuls per [128co x 512yx] PSUM tile (2 ci-tiles
     x 9 shifts), shifted-window APs over a zero-padded 66x66 bf16 image
     loaded in row bands so the PE starts early.
  4. sp map (bf16 matmuls) = style_sp @ (sp_w*ls).T + sp_b, staged to DRAM
     and broadcast back per chunk (bf16). Epilogue per chunk:
     out = (psum * dcol[co,b]) * spmap[yx] in one fused DVE op.

DMA queues: sync(SP-HWDGE) = wt + x bands + outs; scalar(ACT-HWDGE) =
sp_wT chunks + spmap broadcasts + dspt broadcast + sp_b; gpsimd(SWDGE) =
small style pack + mod_b + spmap staging + dspt store.

The baked walrus build only supports ONE sync wait per instruction, so the
Bass subclass rewrites the scheduled BIR JSON, hoisting extra waits onto
single-wait EventSemaphore carriers inserted before the instruction (same
engine => identical blocking semantics).
"""

import json
import sys
from contextlib import ExitStack

for _p in ("/opt/pypackages", "/opt/trn_rl_repo"):
    if _p not in sys.path:
        sys.path.insert(0, _p)

import ml_dtypes
import numpy as np

import concourse.bass as bass
import concourse.mybir as mybir
import concourse.tile as tile
from concourse.bass_utils import run_bass_kernel_spmd

# Problem constants (hardcoded per harness contract)
B, CIN, COUT, K = 32, 256, 256, 3
STYLE_DIM, SP = 512, 64
EPS = 1e-6
LS = 1.0 / (STYLE_DIM // 2) ** 0.5      # EqualLinear scale = 1/16
CS = 1.0 / (CIN * K * K) ** 0.5         # conv fan-in scale = 1/48
N_CORES = 8
BPC = B // N_CORES                      # samples per core = 4
SPP = SP + 2                            # padded image dim = 66
CKK = COUT * K * K                      # 2304 free columns in weight layout
YX = SP * SP                            # 4096 spatial positions

F32 = mybir.dt.float32
BF16 = mybir.dt.bfloat16
AF = mybir.ActivationFunctionType
ALU = mybir.AluOpType

# x row bands per ci-tile: chunk n reads padded rows [8n, 8n+10)
XBANDS = [(0, 18), (18, 42), (42, 66)]


def _split_multi_waits(bir: dict) -> int:
    """Hoist all but one sync wait from every instruction onto single-wait
    EventSemaphore carriers inserted immediately before it (same engine)."""
    ctr = 0
    for fn in bir.get("functions", []):
        for blk in fn.get("blocks", []):
            insts = blk.get("instructions", [])
            if not any(
                len(((i.get("sync_info") or {}).get("on_wait") or [])) > 1
                for i in insts
            ):
                continue
            new_insts = []
            for inst in insts:
                si = inst.get("sync_info")
                ow = (si or {}).get("on_wait") or []
                if len(ow) > 1:
                    for w in ow[:-1]:
                        ctr += 1
                        new_insts.append({
                            "debug": inst.get("debug", 0),
                            "engine": inst["engine"],
                            "ins": [],
                            "outs": [],
                            "name": f"waitsplit-{ctr}",
                            "opcode": "EventSemaphore",
                            "sync_info": {"on_update": [], "on_wait": [w]},
                        })
                    si["on_wait"] = [ow[-1]]
                new_insts.append(inst)
            blk["instructions"] = new_insts
    return ctr


class _WaitSplitBass(bass.Bass):
    def to_json_bytes(self) -> bytes:
        raw = super().to_json_bytes()
        bir = json.loads(raw)
        if _split_multi_waits(bir):
            return json.dumps(bir).encode()
        return raw


def _pbcast(ap, n):
    """Manual 0-step partition broadcast AP (DMA-only; engines reject it)."""
    return bass.AP(tensor=ap.tensor, offset=ap.offset,
                   ap=[[0, n]] + [list(d) for d in ap.ap[1:]])


def _build_program() -> bass.Bass:
    nc = _WaitSplitBass("TRN2", target_bir_lowering=False, debug=False)

    x_d = nc.dram_tensor("x", [BPC, CIN, SPP, SPP], BF16, kind="ExternalInput")
    stylepk_d = nc.dram_tensor("stylepk", [256, 264], F32, kind="ExternalInput")
    wT_d = nc.dram_tensor("wT", [CIN, CKK], BF16, kind="ExternalInput")
    mod_b_d = nc.dram_tensor("mod_b", [CIN, 1], F32, kind="ExternalInput")
    sp_wT_d = nc.dram_tensor("sp_wT", [256, YX], BF16, kind="ExternalInput")
    sp_b_d = nc.dram_tensor("sp_b", [1, YX], F32, kind="ExternalInput")
    out_d = nc.dram_tensor("out", [BPC, COUT, SP, SP], F32, kind="ExternalOutput")
    spm_d = nc.dram_tensor("spm_scratch", [BPC, YX], BF16, kind="Internal")
    dspt_d = nc.dram_tensor("dspt_scratch", [BPC, 1], F32, kind="Internal")

    with tile.TileContext(nc) as tc:
        with tc.tile_pool(name="const", bufs=1) as cpool, \
             tc.tile_pool(name="persist", bufs=1) as ppool, \
             tc.tile_pool(name="sps", bufs=1, space="PSUM") as spsum:

            # ---------- constants ----------
            eps4 = cpool.tile([BPC, 1], F32, name="eps4")
            nc.vector.memset(eps4, EPS)
            eps128 = cpool.tile([128, 1], F32, name="eps128")
            nc.vector.memset(eps128, EPS)
            warm = cpool.tile([128, 640], BF16, name="warm")
            nc.vector.memset(warm, 0.0)

            # ---------- persistent tiles ----------
            wt = [ppool.tile([128, CKK], BF16, name=f"wt{k}") for k in range(2)]
            stylec = [ppool.tile([128, BPC], F32, name=f"stylec{k}") for k in range(2)]
            sts = [ppool.tile([128, BPC], BF16, name=f"sts{k}") for k in range(2)]
            wsqk = [ppool.tile([128, COUT], F32, name=f"wsqk{k}") for k in range(2)]
            dcol_all = [ppool.tile([128, BPC], F32, name=f"dcol{m}") for m in range(2)]
            spm = ppool.tile([BPC, YX], BF16, name="spm")
            spb = ppool.tile([BPC, YX], F32, name="spb")
            sums = ppool.tile([BPC, 8], F32, name="sums")
            dsptb = ppool.tile([128, BPC], F32, name="dsptb")
            stp = [ppool.tile([128, 264], F32, name=f"stp{k}") for k in range(2)]
            mb = [ppool.tile([128, 1], F32, name=f"mb{k}") for k in range(2)]
            mbcs = [ppool.tile([128, 1], F32, name=f"mbcs{k}") for k in range(2)]
            stylec2 = [ppool.tile([128, BPC], F32, name=f"stylec2_{k}") for k in range(2)]

            _stack = ExitStack()
            xppool = _stack.enter_context(tc.tile_pool(name="xp", bufs=2))
            swpool = _stack.enter_context(tc.tile_pool(name="swtc", bufs=1))
            wsqpool = _stack.enter_context(tc.tile_pool(name="wsq", bufs=1))
            scrpool = _stack.enter_context(tc.tile_pool(name="scr", bufs=2))
            cpsum = _stack.enter_context(tc.tile_pool(name="cps", bufs=6, space="PSUM"))
            wscpool = _stack.enter_context(tc.tile_pool(name="wsc", bufs=2))
            opool = _stack.enter_context(tc.tile_pool(name="ot", bufs=8))
            smpool = _stack.enter_context(tc.tile_pool(name="smb", bufs=16))

            # ---------- DMA issue: everything latency-critical on the sync
            # HWDGE queue, smallest-first (SWDGE completion latency is ~5us,
            # so gpsimd carries no loads at all).
            for k in range(2):
                nc.sync.dma_start(out=stp[k], in_=stylepk_d.ap()[k * 128:(k + 1) * 128, :])
            for k in range(2):
                nc.sync.dma_start(out=mb[k], in_=mod_b_d.ap()[k * 128:(k + 1) * 128, :])

            xp0 = [
                xppool.tile([128, SPP * SPP], BF16, name=f"xp{k}_0", tag=f"xp{k}")
                for k in range(2)
            ]

            def load_band(xp, b, bi, k):
                lo, hi = XBANDS[bi]
                nc.sync.dma_start(
                    out=xp[k][:, lo * SPP:hi * SPP],
                    in_=x_d.ap()[b, k * 128:(k + 1) * 128, lo:hi, :]
                        .rearrange("p r c -> p (r c)"),
                )

            swtc = [[None, None] for _ in range(8)]

            def load_swtc(n):
                for k in range(2):
                    t = swpool.tile([128, 512], BF16, name=f"swtc_{n}_{k}", tag=f"swtc{n}_{k}")
                    nc.sync.dma_start(
                        out=t,
                        in_=sp_wT_d.ap()[k * 128:(k + 1) * 128, n * 512:(n + 1) * 512],
                    )
                    swtc[n][k] = t

            nc.sync.dma_start(out=wt[0], in_=wT_d.ap()[0:128, :])
            load_band(xp0, 0, 0, 0)
            load_band(xp0, 0, 0, 1)
            nc.sync.dma_start(out=wt[1], in_=wT_d.ap()[128:256, :])
            load_swtc(0)
            load_swtc(1)
            load_band(xp0, 0, 1, 0)
            load_band(xp0, 0, 1, 1)
            load_swtc(2)
            load_swtc(3)
            load_band(xp0, 0, 2, 0)
            load_band(xp0, 0, 2, 1)
            for n in range(4, 8):
                load_swtc(n)
            nc.scalar.dma_start(out=spb, in_=_pbcast(sp_b_d.ap(), BPC))

            # ---------- PE warm-up: dummy matmuls on zeros keep the HAM
            # activity window busy from ~0.7us so the real stream starts at
            # 2.4GHz instead of 1.2GHz.
            ps_warm = spsum.tile([4, 512], F32, name="ps_warm", tag="ps_sp")
            for _w in range(30):
                nc.tensor.matmul(
                    ps_warm, warm[:, 512:516], warm[:, 0:512],
                    start=True, stop=True, skip_group_check=True,
                )

            # ---------- style path (PE fp32 + DVE epilogue) ----------
            for m in range(2):
                nc.vector.tensor_scalar_mul(mbcs[m], mb[m], CS)
            for k in range(2):
                nc.vector.tensor_copy(sts[k], stp[k][:, 260:264])
            for m in range(2):
                ps_st = spsum.tile([128, BPC], F32, name=f"ps_st{m}", tag="ps_small")
                for k in range(2):
                    nc.tensor.matmul(
                        ps_st, stp[k][:, m * 128:(m + 1) * 128], stp[k][:, 256:260],
                        start=(k == 0), stop=(k == 1),
                    )
                nc.vector.tensor_scalar(
                    out=stylec[m], in0=ps_st, scalar1=LS * CS, scalar2=mbcs[m],
                    op0=ALU.mult, op1=ALU.add,
                )
                nc.scalar.activation(out=stylec2[m], in_=stylec[m], func=AF.Square)

            # ---------- S2 = sum_kk wt^2 (for demod): ACT squares here, the
            # DVE kk-reduces are emitted inside the sample loop AFTER wsc(b0)
            # so DVE's in-order stream doesn't delay the first conv chunk.
            wsq = [wsqpool.tile([128, CKK], F32, name=f"wsq{k}", tag=f"wsq{k}")
                   for k in range(2)]
            for k in range(2):
                nc.scalar.activation(out=wsq[k], in_=wt[k], func=AF.Square)

            # smb tiles per (b, n), loaded by SBUF->SBUF broadcast DMA from spm
            smb_tiles = [[None] * 8 for _ in range(BPC)]

            def load_smb(b, n):
                t = smpool.tile([128, 512], BF16, name=f"smb_{b}_{n}", tag="smb")
                nc.scalar.dma_start(
                    out=t,
                    in_=_pbcast(spm_d.ap()[b:b + 1, n * 512:(n + 1) * 512], 128),
                )
                smb_tiles[b][n] = t

            # ---------- helpers ----------
            def emit_sp_chunk(n):
                """spatial-map chunk n: 2 bf16 MMs -> STT -> bcast + square-acc."""
                ps_sp = spsum.tile([BPC, 512], F32, name=f"ps_sp{n}", tag="ps_sp")
                for k in range(2):
                    nc.tensor.matmul(
                        ps_sp, sts[k], swtc[n][k],
                        start=(k == 0), stop=(k == 1),
                    )
                nc.vector.scalar_tensor_tensor(
                    out=spm[:, n * 512:(n + 1) * 512], in0=ps_sp, scalar=LS,
                    in1=spb[:, n * 512:(n + 1) * 512],
                    op0=ALU.mult, op1=ALU.add,
                )
                nc.scalar.dma_start(
                    out=spm_d.ap()[:, n * 512:(n + 1) * 512],
                    in_=spm[:, n * 512:(n + 1) * 512],
                )
                load_smb(0, n)
                scr = scrpool.tile([BPC, 512], BF16, name=f"scr{n}", tag="scr")
                nc.vector.scalar_tensor_tensor(
                    out=scr, in0=spm[:, n * 512:(n + 1) * 512], scalar=1.0,
                    in1=spm[:, n * 512:(n + 1) * 512],
                    op0=ALU.mult, op1=ALU.mult,
                    accum_out=sums[:, n:n + 1],
                )

            def emit_dspt():
                """global spatial demod scalar -> DRAM -> 128-bcast (both DMAs
                on the scalar queue, which orders store before load)."""
                ssq = ppool.tile([BPC, 1], F32, name="ssq")
                nc.vector.reduce_sum(out=ssq, in_=sums, axis=mybir.AxisListType.X)
                rsq = ppool.tile([BPC, 1], F32, name="rsq")
                nc.vector.reciprocal(rsq, ssq)
                dspt = ppool.tile([BPC, 1], F32, name="dspt")
                nc.scalar.activation(
                    out=dspt, in_=rsq, func=AF.Sqrt, bias=eps4, scale=float(YX),
                )
                nc.scalar.dma_start(out=dspt_d.ap(), in_=dspt)
                nc.scalar.dma_start(
                    out=dsptb,
                    in_=_pbcast(dspt_d.ap().rearrange("b one -> one b"), 128),
                )

            def emit_demod():
                """demodsq[co,b] = S2.T @ stylec^2; dcol = rsqrt(.)*dspt."""
                for m in range(2):
                    ps_d = spsum.tile([128, BPC], F32, name=f"ps_d{m}", tag="ps_small")
                    for k in range(2):
                        nc.tensor.matmul(
                            ps_d, wsqk[k][:, m * 128:(m + 1) * 128], stylec2[k],
                            start=(k == 0), stop=(k == 1),
                        )
                    dsq = ppool.tile([128, BPC], F32, name=f"dsq{m}")
                    nc.scalar.activation(out=dsq, in_=ps_d, func=AF.Sqrt,
                                         bias=eps128, scale=1.0)
                    dinv = ppool.tile([128, BPC], F32, name=f"dinv{m}")
                    nc.vector.reciprocal(dinv, dsq)
                    nc.vector.tensor_mul(dcol_all[m], dinv, dsptb)

            # ---------- per-sample conv pipeline ----------
            xp_tiles = [xp0] + [None] * (BPC - 1)

            def prefetch_xp(b):
                xp = [
                    xppool.tile([128, SPP * SPP], BF16, name=f"xp{k}_{b}", tag=f"xp{k}")
                    for k in range(2)
                ]
                xp_tiles[b] = xp
                return xp

            wsc_tiles = [None] * BPC

            def make_wsc(b):
                ws = [
                    wscpool.tile([128, CKK], BF16, name=f"wsc{k}_{b}", tag=f"wsc{k}")
                    for k in range(2)
                ]
                for k in range(2):
                    nc.vector.tensor_scalar_mul(ws[k], wt[k], stylec[k][:, b:b + 1])
                wsc_tiles[b] = ws

            for b in range(BPC):
                if b == 0:
                    make_wsc(0)
                    # S2 kk-reduces (DVE; lands in its idle window before the
                    # sp-chunk epilogues start)
                    for k in range(2):
                        nc.vector.reduce_sum(
                            out=wsqk[k],
                            in_=wsq[k].rearrange("p (co kk) -> p co kk", kk=9),
                            axis=mybir.AxisListType.X,
                        )
                wsc = wsc_tiles[b]

                xp = xp_tiles[b]
                prefetched = False

                # spmap broadcasts: b==0's are loaded inside emit_sp_chunk;
                # later samples burst at sample start.
                if b > 0:
                    for n in range(8):
                        load_smb(b, n)

                def emit_epilogue(m, n, ps, b=b):
                    ot = opool.tile([128, 512], F32, name=f"ot_{b}_{m}_{n}", tag="ot")
                    nc.vector.scalar_tensor_tensor(
                        out=ot, in0=ps, scalar=dcol_all[m][:, b:b + 1],
                        in1=smb_tiles[b][n],
                        op0=ALU.mult, op1=ALU.mult,
                    )
                    nc.sync.dma_start(
                        out=out_d.ap()[b, m * 128:(m + 1) * 128, n * 8:(n + 1) * 8, :],
                        in_=ot.rearrange("p (r c) -> p r c", c=SP),
                    )

                pending = []  # (m, n, ps) epilogues held until dcol_all exists
                for m in range(2):
                    for n in range(8):
                        ps = cpsum.tile([128, 512], F32, name=f"ps_{b}_{m}_{n}", tag="ps")
                        i = 0
                        for k in range(2):
                            wv = wsc[k].rearrange("p (co kk) -> p co kk", kk=9)
                            xpv = xp[k].rearrange("p (r c) -> p r c", c=SPP)
                            for s in range(9):
                                dy, dx = s // 3, s % 3
                                nc.tensor.matmul(
                                    ps,
                                    wv[:, m * 128:(m + 1) * 128, s],
                                    xpv[:, n * 8 + dy:n * 8 + dy + 8, dx:dx + SP],
                                    start=(i == 0), stop=(i == 17),
                                )
                                i += 1
                        # interleave setup work into sample-0 m=0 PE stream
                        if b == 0 and m == 0:
                            if n < 4:
                                emit_sp_chunk(2 * n)
                                emit_sp_chunk(2 * n + 1)
                                if n == 3:
                                    emit_dspt()
                            elif n == 4:
                                emit_demod()
                        # prefetch next sample's modulated weights in m=0
                        # (keeps DVE's in-order stream from stalling the
                        # sample boundary behind this sample's epilogues)
                        if m == 0 and n == 6 and b + 1 < BPC:
                            make_wsc(b + 1)
                        # prefetch next sample's image in m=1
                        if m == 1 and 1 <= n <= 3 and b + 1 < BPC:
                            if not prefetched:
                                xpn = prefetch_xp(b + 1)
                                prefetched = True
                            load_band(xpn, b + 1, n - 1, 0)
                            load_band(xpn, b + 1, n - 1, 1)
                        # epilogue: for b0/m0 chunks <=4, dcol_all doesn't
                        # exist yet (DVE is in-order) — hold until demod done
                        if b == 0 and m == 0 and n < 4:
                            pending.append((m, n, ps))
                        else:
                            for pm, pn, pps in pending:
                                emit_epilogue(pm, pn, pps)
                            pending = []
                            emit_epilogue(m, n, ps)
            _stack.close()
    return nc


_prog_cache = {}


def _get_program() -> bass.Bass:
    if "nc" not in _prog_cache:
        _prog_cache["nc"] = _build_program()
    return _prog_cache["nc"]


def _make_in_maps(inputs):
    x = np.asarray(inputs["x"], dtype=np.float32)
    x = np.pad(x, ((0, 0), (0, 0), (1, 1), (1, 1))).astype(ml_dtypes.bfloat16)
    style_in = np.asarray(inputs["style_in"], dtype=np.float32)
    weight = np.asarray(inputs["weight"], dtype=np.float32)
    mod_w = np.asarray(inputs["mod_w"], dtype=np.float32)
    mod_b = np.asarray(inputs["mod_b"], dtype=np.float32)
    sp_w = np.asarray(inputs["sp_w"], dtype=np.float32)
    sp_b = np.asarray(inputs["sp_b"], dtype=np.float32)

    # replicated parameter layouts (pure transposes/reshapes + dtype casts)
    wT = np.ascontiguousarray(
        weight[0].transpose(1, 0, 2, 3).reshape(CIN, CKK)).astype(ml_dtypes.bfloat16)
    mod_wT = np.ascontiguousarray(mod_w.T)                        # [sd, ci]
    mod_b2 = np.ascontiguousarray(mod_b.reshape(CIN, 1))
    sp_wT = np.ascontiguousarray(sp_w.T).astype(ml_dtypes.bfloat16)  # [sd, yx]
    sp_b2 = np.ascontiguousarray(sp_b.reshape(1, YX))

    in_maps = []
    for c in range(N_CORES):
        sl = slice(c * BPC, (c + 1) * BPC)
        stylepk = np.concatenate(
            [mod_wT,
             np.ascontiguousarray(style_in[sl, :256].T),
             np.ascontiguousarray(style_in[sl, 256:].T)], axis=1)
        in_maps.append({
            "x": np.ascontiguousarray(x[sl]),
            "stylepk": np.ascontiguousarray(stylepk),
            "wT": wT,
            "mod_b": mod_b2,
            "sp_wT": sp_wT,
            "sp_b": sp_b2,
        })
    return in_maps


def _run(inputs, trace=False):
    nc = _get_program()
    in_maps = _make_in_maps(inputs)
    res = run_bass_kernel_spmd(nc, in_maps, core_ids=list(range(N_CORES)), trace=trace)
    out = np.concatenate([res.results[c]["out"] for c in range(N_CORES)], axis=0)
    return out, res


def kernel(**inputs) -> np.ndarray:
    out, _ = _run(inputs, trace=False)
    return out
